# revision 25
# baseline (speedup 1.0000x reference)
"""Trainium2 Bass kernel for a 12-layer dense transformer encoder
(B=16, T=512, C=1024, H=16, F=4096, V=30522), data-parallel over batch
across 8 NeuronCores (2 sequences = 1024 tokens per core).

v2 design, fully transposed residual stream:
 - The residual x lives TRANSPOSED in SBUF: xT[c, tok] fp32 (8 tiles of
   [128, 1024]).  Every GEMM is emitted "m-outer" so outputs drain
   progressively, and the two 512-token halves (= the 2 sequences) are
   processed as separate pipeline stages so LayerNorm tails overlap the
   other half's matmuls.  No DMA transposes anywhere.
 - LayerNorm stats are computed on the TensorEngine (ones-vector matmuls
   against x in fp32r and x^2 in fp16), the per-token mean/rstd vectors
   with an approx reciprocal, broadcast via GpSimd, applied by DVE.
 - Precision: QK projections and attention*V run fp8 (e4m3) with
   DoubleRow matmuls (2x contraction per pass); exp(scores) is stored
   fp8; everything else runs fp16 operands with fp32 accumulation
   (same TensorE speed as bf16, 4x less rounding noise).
   Weights are pre-scaled by power-of-2 per-tensor factors on the host;
   the scales are folded into biases / activation-scale immediates.
"""
import math
import numpy as np
from contextlib import ExitStack

import ml_dtypes

import concourse.bass as bass
import concourse.mybir as mybir
import concourse.tile as tile
from concourse import bacc

F32 = mybir.dt.float32
F32R = mybir.dt.float32r
F16 = mybir.dt.float16
BF16 = mybir.dt.bfloat16
FP8 = mybir.dt.float8e4
AF = mybir.ActivationFunctionType
ALU = mybir.AluOpType
DR = mybir.MatmulPerfMode.DoubleRow
GELU_AF = AF.Gelu  # sim_test overrides (Gelu not in CoreSim)
AV_FP8 = True   # fp8 sslot/vaug tensors
AV_DR = False   # DoubleRow AV broken on HW for M=65/80 stationaries
DEBUG_DUMP = False  # add intermediate dram outputs (debugging only)

TOK, C, H, D, FF = 1024, 1024, 16, 64, 4096
NCC = C // 128          # 8 chunks of C
NFC = FF // 128         # 32 chunks of F
NKB = C // 256          # 4 DoubleRow contraction blocks over C
NFB = FF // 256         # 16 DoubleRow blocks over F (unused: w2 runs fp16)
EPS = 1e-5
NCORES = 8
EXP_BIAS = -3.0         # exp(z-3): keeps e^z in fp8 range for |z|<~8.4

NP_FP8 = ml_dtypes.float8_e4m3


def _pow2_scale(absmax, target=192.0):
    if absmax <= 0:
        return 1.0
    return float(2.0 ** math.floor(math.log2(target / float(absmax))))


def build_program(layers=12, repeat=1, scales=None):
    """scales: dict with per-layer lists 'swq','swk' (host-chosen pow2
    scale factors baked into immediates)."""
    nc = bacc.Bacc("TRN2", target_bir_lowering=False, debug=False)
    LL = layers
    swq, swk = scales["swq"], scales["swk"]

    x0t = nc.dram_tensor("x0t", (C, TOK), F32, kind="ExternalInput")
    wq8 = nc.dram_tensor("wq8", (LL, NKB, NCC, 128, 2, 128), FP8, kind="ExternalInput")
    wk8 = nc.dram_tensor("wk8", (LL, NKB, NCC, 128, 2, 128), FP8, kind="ExternalInput")
    wv6 = nc.dram_tensor("wv6", (LL, NCC, 2, 128, 512), F16, kind="ExternalInput")
    wo6 = nc.dram_tensor("wo6", (LL, NCC, NCC, 128, 128), F16, kind="ExternalInput")
    w16 = nc.dram_tensor("w16", (LL, NCC, NFC, 128, 128), F16, kind="ExternalInput")
    w26 = nc.dram_tensor("w26", (LL, NFC, NCC, 128, 128), F16, kind="ExternalInput")
    bqs = nc.dram_tensor("bqs", (LL, NCC, 128), F32, kind="ExternalInput")
    bks = nc.dram_tensor("bks", (LL, NCC, 128), F32, kind="ExternalInput")
    bvv = nc.dram_tensor("bvv", (LL, C), F16, kind="ExternalInput")
    bob = nc.dram_tensor("bob", (LL, NCC, 128), F32, kind="ExternalInput")
    b1b = nc.dram_tensor("b1b", (LL, NFC, 128), F32, kind="ExternalInput")
    b2b = nc.dram_tensor("b2b", (LL, NCC, 128), F32, kind="ExternalInput")
    hwp = nc.dram_tensor("hwp", (NCC, 128), F16, kind="ExternalInput")
    hcst = nc.dram_tensor("hcst", (1, 2), F32, kind="ExternalInput")  # [swp, hb]
    yo = nc.dram_tensor("y", (1, TOK), F32, kind="ExternalOutput")
    if DEBUG_DUMP:
        dbg_h8 = nc.dram_tensor("dbg_h8", (NKB, 128, 2, TOK), FP8,
                                kind="ExternalOutput")
        dbg_hh = nc.dram_tensor("dbg_hh", (NCC, 128, TOK), F16,
                                kind="ExternalOutput")
        dbg_q = nc.dram_tensor("dbg_q", (NCC, 128, TOK), F16,
                               kind="ExternalOutput")
        dbg_k = nc.dram_tensor("dbg_k", (NCC, 128, TOK), F16,
                               kind="ExternalOutput")
        dbg_va = nc.dram_tensor("dbg_va", (4, 128, 2, H, D + 1), FP8,
                                kind="ExternalOutput")
        dbg_y = nc.dram_tensor("dbg_y", (NCC, 128, TOK), F16,
                               kind="ExternalOutput")
        dbg_x = nc.dram_tensor("dbg_x", (NCC, 128, TOK), F32,
                               kind="ExternalOutput")
        dbg_ss = nc.dram_tensor("dbg_ss", (2, 128, 2, 512), FP8,
                                kind="ExternalOutput")
        dbg_rz = nc.dram_tensor("dbg_rz", (1, 512), F32,
                                kind="ExternalOutput")

    with tile.TileContext(nc) as tc, ExitStack() as ctx:
        px = ctx.enter_context(tc.tile_pool(name="px", bufs=1))
        pwv = ctx.enter_context(tc.tile_pool(name="pwv", bufs=1))
        pws = ctx.enter_context(tc.tile_pool(name="pws", bufs=5))
        pwm = ctx.enter_context(tc.tile_pool(name="pwm", bufs=5))
        psl = ctx.enter_context(tc.tile_pool(name="psl", bufs=3))
        pbias = ctx.enter_context(tc.tile_pool(name="pbias", bufs=1))
        pmisc = ctx.enter_context(tc.tile_pool(name="pmisc", bufs=2))
        psmall = ctx.enter_context(tc.tile_pool(name="psmall", bufs=1))
        pbig = ctx.enter_context(tc.tile_pool(name="pbig", bufs=2, space="PSUM"))
        pp2 = ctx.enter_context(tc.tile_pool(name="pp2", bufs=2, space="PSUM"))
        pst = ctx.enter_context(tc.tile_pool(name="pst", bufs=1, space="PSUM"))

        # ---- persistent SBUF state ----
        xs = [px.tile([128, TOK], F32, tag=f"x{m}", name=f"x{m}") for m in range(NCC)]
        qT = [px.tile([128, TOK], F16, tag=f"q{m}", name=f"q{m}") for m in range(NCC)]
        kTt = [px.tile([128, TOK], F16, tag=f"k{m}", name=f"k{m}") for m in range(NCC)]
        yT = [px.tile([128, TOK], F16, tag=f"y{m}", name=f"y{m}") for m in range(NCC)]
        # fp8 DoubleRow-interleaved LN1 output (for QK):  [p, i, tok]
        h8 = [px.tile([128, 2, TOK], FP8, tag=f"h8{j}", name=f"h8{j}")
              for j in range(NKB)]
        # fp16 LN output (h for V, then h2 for W1 — sequential reuse)
        hh = [px.tile([128, TOK], F16, tag=f"hh{m}", name=f"hh{m}")
              for m in range(NCC)]
        # fp16 gelu output, per-half (reused h0 -> h1)
        gT = [px.tile([128, 512], F16, tag=f"g{m}", name=f"g{m}")
              for m in range(NFC)]
        # fp8 v, DoubleRow pairs: [p, i, head, d+1]
        VW = D + 1  # v columns + ones column (AV runs plain matmuls)
        va = [px.tile([128, 2, H, VW], FP8 if AV_FP8 else F16,
                      tag=f"va{j}", name=f"va{j}")
              for j in range(4)]

        ones_h = px.tile([128, 1], F16, tag="onesh", name="ones_h")
        nc.vector.memset(ones_h[:], 1.0)
        hw_sb = px.tile([128, NCC], F16, tag="hwsb", name="hw_sb")
        nc.sync.dma_start(hw_sb[:], hwp.ap().rearrange("m p -> p m"))
        hc_sb = px.tile([1, 2], F32, tag="hcsb", name="hc_sb")
        nc.sync.dma_start(hc_sb[:], hcst.ap())
        z128 = px.tile([128, 1], F32, tag="z128", name="z128")
        nc.vector.memset(z128[:], 0.0)
        eb128 = px.tile([128, 1], F32, tag="eb128", name="eb128")
        nc.vector.memset(eb128[:], EXP_BIAS)
        eps1 = px.tile([1, 1], F32, tag="eps1", name="eps1")
        nc.vector.memset(eps1[:], EPS)
        z1 = px.tile([1, 1], F32, tag="z1", name="z1")
        nc.vector.memset(z1[:], 0.0)

        HS = [slice(0, 512), slice(512, 1024)]

        def ln_stats_mm(stats, m, half, first, last, head_zp=None):
            """accumulate Sum(x) into stats[,:512], Sum(x^2) into stats[,512:]
            (x is copied to f16 for the TensorE ones-matmuls)"""
            xsl = xs[m][:, HS[half]]
            x16 = pmisc.tile([128, 512], F16, tag="x16", name="x16")
            nc.vector.tensor_copy(x16[:], xsl)
            sq_t = pmisc.tile([128, 512], F16, tag="sq", name="sq_t")
            nc.scalar.activation(sq_t[:], xsl, AF.Square, bias=z128[:])
            nc.tensor.matmul(stats[0:1, 0:512], ones_h[:], x16[:],
                             start=first, stop=last)
            nc.tensor.matmul(stats[0:1, 512:1024], ones_h[:], sq_t[:],
                             start=first, stop=last)
            if head_zp is not None:
                nc.tensor.matmul(head_zp[:], hw_sb[:, m:m + 1], x16[:],
                                 start=first, stop=last)

        def ln_tail(stats, half, dst8, dst16, keep=False):
            """compute mean/rstd from stats psum, broadcast, apply.
            dst8: list of 4 [128,2,TOK] fp8 tiles or None
            dst16: list of 8 [128,TOK] f16 tiles or None
            keep: return (m, rz, mr) [1,512] tiles for the head"""
            ma = psmall.tile([1, 512], F32, tag="lna", name="ln_m")
            sc = psmall.tile([1, 512], F32, tag="lnb", name="ln_sc")
            rzt = psmall.tile([1, 512], F32, tag="lnc", name="ln_rz")
            m_sb, rz = ma[:], rzt[:]
            nc.vector.tensor_scalar(m_sb, stats[0:1, 0:512], 1.0 / C, None,
                                    ALU.mult)
            nc.vector.tensor_tensor(sc[:], m_sb, m_sb, ALU.mult)
            nc.vector.scalar_tensor_tensor(sc[:], stats[0:1, 512:1024],
                                           1.0 / C, sc[:], ALU.mult,
                                           ALU.subtract)
            nc.scalar.activation(sc[:], sc[:], AF.Sqrt, bias=eps1[:])
            nc.vector.reciprocal_approx_fast(rz, sc[:])
            # m <- m*rz  (mr), in place
            nc.vector.tensor_tensor(ma[:], ma[:], rz, ALU.mult)
            mr = ma[:]
            if dst8 is None and dst16 is None:
                return (rz, mr) if keep else None
            rz_b = pmisc.tile([128, 512], F32, tag="rzb", name="rz_b")
            nc.gpsimd.partition_broadcast(rz_b[:], rz)
            mr_b = pmisc.tile([128, 512], F32, tag="mrb", name="mr_b")
            nc.gpsimd.partition_broadcast(mr_b[:], mr)
            for m in range(NCC):
                t1 = pmisc.tile([128, 512], F16, tag="t1", name="t1")
                nc.vector.tensor_tensor(t1[:], xs[m][:, HS[half]], rz_b[:],
                                        ALU.mult)
                if dst16 is not None:
                    nc.vector.tensor_tensor(dst16[m][:, HS[half]], t1[:],
                                            mr_b[:], ALU.subtract)
                if dst8 is not None:
                    nc.vector.tensor_tensor(dst8[m // 2][:, m % 2, HS[half]],
                                            t1[:], mr_b[:], ALU.subtract)
            if keep:
                return rz, mr
            return None

        def emit_qk(l, half, m, bq_sb, bk_sb):
            pqk = pbig.tile([128, 1024], F32, tag="sc", name="pqk")
            pq, pk = pqk[:, 0:512], pqk[:, 512:1024]
            for kb in range(NKB):
                cq = pws.tile([128, 2, 128], FP8, tag="cq", name="cq")
                nc.sync.dma_start(cq[:], wq8.ap()[l, kb, m])
                ck = pws.tile([128, 2, 128], FP8, tag="ck", name="ck")
                nc.sync.dma_start(ck[:], wk8.ap()[l, kb, m])
                st, sp = (kb == 0), (kb == NKB - 1)
                nc.tensor.matmul(pq, cq[:], h8[kb][:, :, HS[half]],
                                 start=st, stop=sp, perf_mode=DR)
                nc.tensor.matmul(pk, ck[:], h8[kb][:, :, HS[half]],
                                 start=st, stop=sp, perf_mode=DR)
            nc.vector.tensor_scalar(qT[m][:, HS[half]], pq,
                                    bq_sb[:, m:m + 1], None, ALU.add)
            nc.vector.tensor_scalar(kTt[m][:, HS[half]], pk,
                                    bk_sb[:, m:m + 1], None, ALU.add)

        def emit_v(l, t, wv_sb, bv_b):
            """v projection for token chunk t -> vaug pair j=t//2 slot t%2"""
            pvs = []
            for blk in range(2):
                pv = pp2.tile([128, 512], F32, tag="pp", name="pv")
                pvs.append(pv)
            for kc in range(NCC):
                hsl = hh[kc][:, t * 128:(t + 1) * 128]
                for blk in range(2):
                    nc.tensor.matmul(pvs[blk][:], hsl, wv_sb[kc][blk][:],
                                     start=(kc == 0), stop=(kc == NCC - 1))
            j, i = t // 2, t % 2
            for blk in range(2):
                nc.vector.tensor_tensor(
                    va[j][:, i, blk * 8:(blk + 1) * 8, 0:D],
                    pvs[blk][:].rearrange("p (h d) -> p h d", h=8),
                    bv_b[blk][:].rearrange("p (h d) -> p h d", h=8),
                    ALU.add)

        def emit_scores(l, b, h, exp_scale):
            ct, r0 = h // 2, 64 * (h % 2)
            ksl = kTt[ct][r0:r0 + 64, HS[b]]
            qsl = qT[ct][r0:r0 + 64, HS[b]]
            sss = []
            for p in range(2):
                pp = pbig.tile([128, 1024], F32, tag="sc", name="psc")
                for i in range(2):
                    kk = 2 * p + i
                    nc.tensor.matmul(pp[:, i * 512:(i + 1) * 512],
                                     ksl[:, kk * 128:(kk + 1) * 128], qsl,
                                     start=True, stop=True)
                ss = psl.tile([128, 2, 512], FP8 if AV_FP8 else F16,
                              tag="ss", name="ss")
                nc.scalar.activation(ss[:].rearrange("p a b -> p (a b)"),
                                     pp[:], AF.Exp, bias=eb128[:],
                                     scale=exp_scale)
                sss.append(ss)
            return sss

        def emit_av(l, b, h, sss):
            ct, r0 = h // 2, 64 * (h % 2)
            py = pp2.tile([VW, 512], F32, tag="pp", name="py")
            if AV_DR:
                for p in range(2):
                    nc.tensor.matmul(py[:], va[b * 2 + p][:, :, h, :],
                                     sss[p][:], start=(p == 0), stop=(p == 1),
                                     perf_mode=DR)
            else:
                for p in range(2):
                    for i in range(2):
                        nc.tensor.matmul(py[:], va[b * 2 + p][:, i, h, :],
                                         sss[p][:, i, :],
                                         start=(p == 0 and i == 0),
                                         stop=(p == 1 and i == 1))
            if DEBUG_DUMP and l == 0 and b == 0 and h == 0:
                for p in range(2):
                    nc.sync.dma_start(dbg_ss.ap()[p], sss[p][:])
            # custom-DVE ops drop non-zero start partitions on HW: stage the
            # denominator row to partition 0 with a regular copy first
            dnm = pmisc.tile([1, 512], F32, tag="dnm", name="dnm")
            nc.vector.tensor_copy(dnm[:], py[64:65, :])
            rz1 = pmisc.tile([1, 512], F32, tag="rza", name="rz1")
            nc.vector.reciprocal_approx_fast(rz1[:], dnm[:])
            if DEBUG_DUMP and l == 0 and b == 0 and h == 0:
                nc.sync.dma_start(dbg_rz.ap(), rz1[:])
            rzr = pmisc.tile([128, 512], F32, tag="rzr", name="rzr")
            nc.gpsimd.partition_broadcast(rzr[:], rz1[:])
            nc.vector.tensor_tensor(yT[ct][r0:r0 + 64, HS[b]], py[0:64, :],
                                    rzr[r0:r0 + 64, :], ALU.mult)

        def emit_wstat(l, half, m, wdram, nk, moving, bias_sb, stats, sfirst,
                       slice_half=True, head_zp=None):
            """generic W-stationary fp16 GEMM with residual-add epilogue and
            LN-stats interleave.  moving: list of [128,TOK]|[128,512] tiles"""
            ps = pp2.tile([128, 512], F32, tag="pp", name="pws")
            for kc in range(nk):
                wt = pwm.tile([128, 128], F16, tag="wm", name="wt")
                nc.sync.dma_start(wt[:], wdram.ap()[l, kc, m])
                mv = moving[kc][:, HS[half]] if slice_half else moving[kc][:]
                nc.tensor.matmul(ps[:], wt[:], mv,
                                 start=(kc == 0), stop=(kc == nk - 1))
            xsl = xs[m][:, HS[half]]
            nc.vector.tensor_scalar(xsl, xsl, bias_sb[:, m:m + 1], None,
                                    ALU.add)
            nc.vector.scalar_tensor_tensor(xsl, ps[:], 1.0, xsl,
                                           ALU.mult, ALU.add)
            if stats is not None:
                ln_stats_mm(stats, m, half, sfirst, m == NCC - 1,
                            head_zp=head_zp)

        def emit_w1(l, half, ft, b1_sb):
            ps = pp2.tile([128, 512], F32, tag="pp", name="pw1")
            for kc in range(NCC):
                wt = pwm.tile([128, 128], F16, tag="wm", name="w1t")
                nc.sync.dma_start(wt[:], w16.ap()[l, kc, ft])
                nc.tensor.matmul(ps[:], wt[:], hh[kc][:, HS[half]],
                                 start=(kc == 0), stop=(kc == NCC - 1))
            nc.scalar.activation(gT[ft][:], ps[:], GELU_AF,
                                 bias=b1_sb[:, ft:ft + 1])

        y_sb = px.tile([1, TOK], F32, tag="ysb", name="y_sb")

        def emit_head(half, rzf, mrf, zp):
            """z = rz*(x.w') + mr*(-swp) + hb ; y = ln(1+exp(z))
            (the zp psum was accumulated during the final stats loop)"""
            et = psmall.tile([1, 512], F32, tag="lnd", name="head_e")
            nc.vector.tensor_tensor(et[:], zp[:], rzf, ALU.mult)
            nc.vector.scalar_tensor_tensor(et[:], mrf, hc_sb[0:1, 0:1],
                                           et[:], ALU.mult, ALU.add)
            nc.scalar.activation(et[:], et[:], AF.Exp, bias=hc_sb[0:1, 1:2])
            nc.vector.tensor_scalar(et[:], et[:], 1.0, None, ALU.add)
            nc.scalar.activation(y_sb[0:1, HS[half]], et[:], AF.Ln,
                                 bias=z1[:])

        def whole_net(_iv=None):
            for m in range(NCC):
                nc.sync.dma_start(xs[m][:], x0t.ap()[m * 128:(m + 1) * 128, :])
            # prologue LN1 for both halves
            for half in range(2):
                stats = pst.tile([1, 1024], F32, tag="st", name="st0")
                for m in range(NCC):
                    ln_stats_mm(stats, m, half, m == 0, m == NCC - 1)
                ln_tail(stats, half, h8, hh)

            for l in range(layers):
                exp_scale = 1.0 / (swq[l] * swk[l])
                # per-layer bias tiles
                bq_sb = pbias.tile([128, NCC], F32, tag="bq", name="bq_sb")
                nc.sync.dma_start(bq_sb[:], bqs.ap()[l].rearrange("m p -> p m"))
                bk_sb = pbias.tile([128, NCC], F32, tag="bk", name="bk_sb")
                nc.sync.dma_start(bk_sb[:], bks.ap()[l].rearrange("m p -> p m"))
                bo_sb = pbias.tile([128, NCC], F32, tag="bo", name="bo_sb")
                nc.sync.dma_start(bo_sb[:], bob.ap()[l].rearrange("m p -> p m"))
                b1_sb = pbias.tile([128, NFC], F32, tag="b1", name="b1_sb")
                nc.sync.dma_start(b1_sb[:], b1b.ap()[l].rearrange("m p -> p m"))
                b2_sb = pbias.tile([128, NCC], F32, tag="b2", name="b2_sb")
                nc.sync.dma_start(b2_sb[:], b2b.ap()[l].rearrange("m p -> p m"))
                bv_b = []
                for blk in range(2):
                    bvt = pbias.tile([128, 512], F16, tag=f"bv{blk}",
                                     name=f"bv_b{blk}")
                    nc.sync.dma_start(
                        bvt[:],
                        bvv.ap()[l:l + 1, blk * 512:(blk + 1) * 512]
                        .to_broadcast((128, 512)))
                    bv_b.append(bvt)
                wv_sb = []
                for kc in range(NCC):
                    pair = []
                    for blk in range(2):
                        wvt = pwv.tile([128, 512], F16, tag=f"wv{kc}_{blk}",
                                       name=f"wv{kc}_{blk}")
                        nc.sync.dma_start(wvt[:], wv6.ap()[l, kc, blk])
                        pair.append(wvt)
                    wv_sb.append(pair)

                # ones for vaug (per layer, tiles are persistent)
                if l == 0:
                    for j in range(4):
                        nc.vector.memset(va[j][:, :, :, D:D + 1], 1.0)

                if DEBUG_DUMP and l == 0:
                    for j in range(NKB):
                        nc.sync.dma_start(dbg_h8.ap()[j], h8[j][:])
                    for m in range(NCC):
                        nc.sync.dma_start(dbg_hh.ap()[m], hh[m][:])
                for m in range(NCC):
                    emit_qk(l, 0, m, bq_sb, bk_sb)
                for m in range(NCC):
                    emit_qk(l, 1, m, bq_sb, bk_sb)
                for t in range(4):
                    emit_v(l, t, wv_sb, bv_b)

                if DEBUG_DUMP and l == 0:
                    for m in range(NCC):
                        nc.sync.dma_start(dbg_q.ap()[m], qT[m][:])
                        nc.sync.dma_start(dbg_k.ap()[m], kTt[m][:])
                # attention batch 0, software-pipelined one unit deep,
                # V chunks t=4..7 as PE filler
                prev = None
                for h in range(H):
                    cur = emit_scores(l, 0, h, exp_scale)
                    if prev is not None:
                        emit_av(l, 0, h - 1, prev)
                    if h % 4 == 3:
                        emit_v(l, 4 + h // 4, wv_sb, bv_b)
                    prev = cur
                emit_av(l, 0, H - 1, prev)

                if DEBUG_DUMP and l == 0:
                    for j in range(4):
                        nc.sync.dma_start(dbg_va.ap()[j], va[j][:])
                # attention batch 1 with Wo(half0) chunks as PE filler
                stats20 = pst.tile([1, 1024], F32, tag="st", name="st20")
                wo_m = 0
                prev = None
                for h in range(H):
                    cur = emit_scores(l, 1, h, exp_scale)
                    if prev is not None:
                        emit_av(l, 1, h - 1, prev)
                    if h % 2 == 1:
                        emit_wstat(l, 0, wo_m, wo6, NCC, yT, bo_sb,
                                   stats20, wo_m == 0)
                        wo_m += 1
                    prev = cur
                emit_av(l, 1, H - 1, prev)
                while wo_m < NCC:
                    emit_wstat(l, 0, wo_m, wo6, NCC, yT, bo_sb,
                               stats20, wo_m == 0)
                    wo_m += 1
                ln_tail(stats20, 0, None, hh)

                if DEBUG_DUMP and l == 0:
                    for m in range(NCC):
                        nc.sync.dma_start(dbg_y.ap()[m], yT[m][:])
                stats21 = pst.tile([1, 1024], F32, tag="st", name="st21")
                for m in range(NCC):
                    emit_wstat(l, 1, m, wo6, NCC, yT, bo_sb, stats21, m == 0)
                ln_tail(stats21, 1, None, hh)

                last = l == layers - 1
                for ft in range(NFC):
                    emit_w1(l, 0, ft, b1_sb)
                stats10 = pst.tile([1, 1024], F32, tag="st", name="st10")
                zp0 = pp2.tile([1, 512], F32, tag="pp", name="zp0") \
                    if last else None
                for m in range(NCC):
                    emit_wstat(l, 0, m, w26, NFC, gT, b2_sb, stats10, m == 0,
                               slice_half=False, head_zp=zp0)
                if not last:
                    ln_tail(stats10, 0, h8, hh)
                else:
                    rzf, mrf = ln_tail(stats10, 0, None, None, keep=True)
                    emit_head(0, rzf, mrf, zp0)

                for ft in range(NFC):
                    emit_w1(l, 1, ft, b1_sb)
                stats11 = pst.tile([1, 1024], F32, tag="st", name="st11")
                zp1 = pp2.tile([1, 512], F32, tag="pp", name="zp1") \
                    if last else None
                for m in range(NCC):
                    emit_wstat(l, 1, m, w26, NFC, gT, b2_sb, stats11, m == 0,
                               slice_half=False, head_zp=zp1)
                if not last:
                    ln_tail(stats11, 1, h8, hh)
                else:
                    rzf, mrf = ln_tail(stats11, 1, None, None, keep=True)
                    emit_head(1, rzf, mrf, zp1)

            if DEBUG_DUMP:
                for m in range(NCC):
                    nc.sync.dma_start(dbg_x.ap()[m], xs[m][:])
            nc.sync.dma_start(yo.ap(), y_sb[:])

        if repeat == 1:
            whole_net()
        else:
            tc.For_i_unrolled(0, repeat, 1, whole_net, max_unroll=1)

    nc.compile()
    return nc


def prep_weights(inputs, layers=12):
    f16 = np.float16
    f32 = np.float32

    ln1_w = np.asarray(inputs["ln1_w"], f32)
    ln1_b = np.asarray(inputs["ln1_b"], f32)
    ln2_w = np.asarray(inputs["ln2_w"], f32)
    ln2_b = np.asarray(inputs["ln2_b"], f32)

    L = layers
    scale = f32(1.0) / np.sqrt(f32(D))

    wq8 = np.empty((L, NKB, NCC, 128, 2, 128), NP_FP8)
    wk8 = np.empty((L, NKB, NCC, 128, 2, 128), NP_FP8)
    wv6 = np.empty((L, NCC, 2, 128, 512), f16)
    wo6 = np.empty((L, NCC, NCC, 128, 128), f16)
    w16 = np.empty((L, NCC, NFC, 128, 128), f16)
    w26 = np.empty((L, NFC, NCC, 128, 128), f16)
    bqs = np.empty((L, NCC, 128), f32)
    bks = np.empty((L, NCC, 128), f32)
    bvv = np.empty((L, C), f16)
    bob = np.empty((L, NCC, 128), f32)
    b1b = np.empty((L, NFC, 128), f32)
    b2b = np.empty((L, NCC, 128), f32)
    swq, swk = [], []

    def dr_tile(w, s):
        # [C_in, M] -> [C_in/256, M/128, 128, 2, 128] fp8 with scale s
        ci, mm = w.shape
        t = (w * s).reshape(ci // 256, 2, 128, mm // 128, 128)
        t = t.transpose(0, 3, 2, 1, 4)  # kb, m, p, i, c
        return np.clip(t, -240.0, 240.0).astype(NP_FP8)

    def stat16(w):
        # [K, M] -> [K/128, M/128, 128, 128] f16
        k, mm = w.shape
        return np.ascontiguousarray(
            w.reshape(k // 128, 128, mm // 128, 128).transpose(0, 2, 1, 3)
        ).astype(f16)

    for l in range(L):
        Wq = np.asarray(inputs["Wq"][l], f32)
        Wk = np.asarray(inputs["Wk"][l], f32)
        Wv = np.asarray(inputs["Wv"][l], f32)
        Wo = np.asarray(inputs["Wo"][l], f32)
        W1 = np.asarray(inputs["W1"][l], f32)
        W2 = np.asarray(inputs["W2"][l], f32)
        d1 = ln1_w[l][:, None]
        d2 = ln2_w[l][:, None]

        Wq_f = (d1 * Wq) * scale
        Wk_f = d1 * Wk
        Wv_f = d1 * Wv
        W1_f = d2 * W1

        sq = _pow2_scale(np.abs(Wq_f).max())
        sk = _pow2_scale(np.abs(Wk_f).max())
        swq.append(sq)
        swk.append(sk)
        wq8[l] = dr_tile(Wq_f, sq)
        wk8[l] = dr_tile(Wk_f, sk)
        # v moving: [C_in, C_out] -> [kc, blk, 128, 512]
        wv6[l] = np.ascontiguousarray(
            Wv_f.reshape(NCC, 128, 2, 512).transpose(0, 2, 1, 3)).astype(f16)
        wo6[l] = stat16(Wo)
        w16[l] = stat16(W1_f)
        w26[l] = stat16(W2)

        bqs[l] = (sq * (ln1_b[l] @ Wq + np.asarray(inputs["bq"][l], f32))
                  * scale).reshape(NCC, 128)
        bks[l] = (sk * (ln1_b[l] @ Wk + np.asarray(inputs["bk"][l], f32))
                  ).reshape(NCC, 128)
        bvv[l] = ln1_b[l] @ Wv + np.asarray(inputs["bv"][l], f32)
        bob[l] = np.asarray(inputs["bo"][l], f32).reshape(NCC, 128)
        b1b[l] = (ln2_b[l] @ W1 + np.asarray(inputs["b1"][l], f32)
                  ).reshape(NFC, 128)
        b2b[l] = np.asarray(inputs["b2"][l], f32).reshape(NCC, 128)

    head_w = np.asarray(inputs["head_w"], f32)  # [C, 1]
    wprime = (np.asarray(inputs["ln_f_w"], f32) * head_w[:, 0])  # [C]
    swp = float(wprime.sum())
    hb = float(np.asarray(inputs["ln_f_b"], f32) @ head_w[:, 0]
               + np.asarray(inputs["head_b"], f32)[0])

    out = {
        "wq8": wq8, "wk8": wk8, "wv6": wv6, "wo6": wo6, "w16": w16,
        "w26": w26, "bqs": bqs, "bks": bks, "bvv": bvv, "bob": bob,
        "b1b": b1b, "b2b": b2b,
        "hwp": np.ascontiguousarray(wprime.reshape(NCC, 128)).astype(f16),
        # host negates swp so the in-kernel stt op computes  tz + mr*(-swp)
        "hcst": np.array([[-swp, hb]], f32),
    }
    return out, {"swq": swq, "swk": swk}


def prep_x0(inputs):
    idx = np.asarray(inputs["idx"])
    tok = np.asarray(inputs["tok_emb"], np.float32)
    pos = np.asarray(inputs["pos_emb"], np.float32)
    x0 = tok[idx] + pos  # [B, T, C]
    outs = []
    for c in range(NCORES):
        xc = x0[2 * c:2 * c + 2].reshape(TOK, C)
        outs.append(np.ascontiguousarray(xc.T))  # [C, TOK]
    return outs


class SpmdRunner:
    """Executes a compiled Bass module on the 8 axon-attached NeuronCores via
    PJRT (modeled on concourse.bass2jax.run_bass_via_pjrt, but jits once and
    keeps inputs device-resident so repeated calls are cheap)."""

    def __init__(self, nc, n_cores=NCORES):
        import jax
        from jax.sharding import Mesh, PartitionSpec
        from jax.experimental.shard_map import shard_map
        from concourse import bass2jax
        from concourse.bass2jax import _bass_exec_p, install_neuronx_cc_hook

        install_neuronx_cc_hook()
        self.jax = jax
        self.nc = nc
        self.n_cores = n_cores
        self.PartitionSpec = PartitionSpec

        partition_name = (
            nc.partition_id_tensor.name if nc.partition_id_tensor else None)
        in_names, out_names, out_avals = [], [], []
        self.extra_zero_names = []
        for alloc in nc.m.functions[0].allocations:
            if not isinstance(alloc, mybir.MemoryLocationSet):
                continue
            name = alloc.memorylocations[0].name
            if alloc.kind == "ExternalInput":
                if name != partition_name:
                    in_names.append(name)
            elif alloc.kind == "ExternalOutput":
                out_names.append(name)
                out_avals.append(jax.core.ShapedArray(
                    tuple(alloc.tensor_shape), mybir.dt.np(alloc.dtype)))
        if nc.dbg_addr is not None:
            self.extra_zero_names.append(nc.dbg_addr.name)

        self.in_names = list(in_names)
        self.out_names = out_names
        self.out_avals = out_avals
        n_params = len(in_names) + len(self.extra_zero_names)
        n_outs = len(out_avals)
        all_in_names = list(in_names) + self.extra_zero_names + list(out_names)
        if partition_name is not None:
            all_in_names.append(partition_name)

        def _body(*args):
            operands = list(args)
            if partition_name is not None:
                operands.append(bass2jax.partition_id_tensor())
            outs = _bass_exec_p.bind(
                *operands,
                out_avals=tuple(out_avals),
                in_names=tuple(all_in_names),
                out_names=tuple(out_names),
                lowering_input_output_aliases=(),
                sim_require_finite=True,
                sim_require_nnan=True,
                nc=nc,
            )
            return tuple(outs)

        devices = jax.devices()[:n_cores]
        assert len(devices) == n_cores, (
            f"need {n_cores} neuron cores, found {len(devices)}")
        self.mesh = Mesh(np.asarray(devices), ("core",))
        in_specs = (PartitionSpec("core"),) * (n_params + n_outs)
        out_specs = (PartitionSpec("core"),) * n_outs
        self.fn = jax.jit(
            shard_map(_body, mesh=self.mesh, in_specs=in_specs,
                      out_specs=out_specs, check_rep=False),
            keep_unused=True)
        self._dev_args = None

    def place_inputs(self, in_maps):
        jax = self.jax
        sharding = jax.sharding.NamedSharding(
            self.mesh, self.PartitionSpec("core"))
        args = []
        for name in self.in_names:
            concat = np.concatenate(
                [np.asarray(in_maps[c][name]) for c in range(self.n_cores)],
                axis=0)
            args.append(jax.device_put(concat, sharding))
        for name in self.extra_zero_names:
            args.append(jax.device_put(
                np.zeros((self.n_cores, 2), np.uint32), sharding))
        for aval in self.out_avals:
            args.append(jax.device_put(
                np.zeros((self.n_cores * aval.shape[0], *aval.shape[1:]),
                         aval.dtype), sharding))
        self._dev_args = args

    def run(self):
        outs = self.fn(*self._dev_args)
        self.jax.block_until_ready(outs)
        return outs

    def results(self, outs):
        per_core = []
        for c in range(self.n_cores):
            d = {}
            for i, name in enumerate(self.out_names):
                aval = self.out_avals[i]
                d[name] = np.asarray(outs[i]).reshape(
                    self.n_cores, *aval.shape)[c]
            per_core.append(d)
        return per_core


_CACHE = {}


def _get_runner(repeat=1, scales=None):
    key = ("prog", repeat)
    if key not in _CACHE:
        ncb = build_program(layers=12, repeat=repeat, scales=scales)
        _CACHE[key] = SpmdRunner(ncb, NCORES)
    return _CACHE[key]


def kernel(**inputs) -> np.ndarray:
    w, scales = prep_weights(inputs, layers=12)
    x0s = prep_x0(inputs)
    runner = _get_runner(repeat=1, scales=scales)
    in_maps = [dict(w, x0t=x0s[c]) for c in range(NCORES)]
    runner.place_inputs(in_maps)
    outs = runner.run()
    res = runner.results(outs)
    y = np.stack([res[c]["y"].reshape(2, 512) for c in range(NCORES)])
    return np.ascontiguousarray(y.reshape(16, 512).astype(np.float32))
